# revision 51
# baseline (speedup 1.0000x reference)
"""Multi-head causal attention (B=4, S=2048, H=16, hd=64) on 8 TRN2 NeuronCores.

Sharding: core c handles batch b = c//2 and head-group g = c%2 (8 heads each,
i.e. columns g*512:(g+1)*512 of the 1024-wide vec dim). Each core computes its
heads' attention context and a partial output projection against its 512
columns of w_out; the host sums the two partials per batch (the row-parallel
all-reduce done on host, since the contract is full-I/O anyway).

Per-core kernel layout (everything transposed: [feature, seq] on partitions):
  xT[512, 2048] -> per head-pair x2hT[128, 2048]
  qT/kT = blockdiag(W^T) @ x2hT            [128, 2048]  (2 heads packed)
  v_aug[128, 2, 16, 65] with a ones column -> row 64 of ctx psum
          accumulates the softmax denominator for free
  scores: per pair, heads A and B issued as adjacent row-tiled matmuls
          (tile_position (0,0) and (64,0)) draining into different PSUM banks
          of one [128,1024] staging tile -> concurrent, ~2x scores rate
  E = exp(scores/8) in ONE activation per staging tile (bf16 out)
  ctxT[65, QB] += v_aug_tile^T @ E_tile   over k-tiles (M=65: +denominator)
  normalize: reciprocal(denom row) -> gpsimd partition_broadcast -> fused mul
  out[s-tile, 1024] = sum_p ctx_chunk_p^T @ woT_chunk_p  (deferred one q-block,
          interleaved into the next block's scores to keep PE dense)
Matmuls run in float32r (TF32-like, 1 cycle/col at N>=256); E is bf16.
"""
import os
import sys
from collections import deque

for _p in ("/opt/trn_rl_repo", "/root/.axon_site/_ro/trn_rl_repo"):
    if os.path.isdir(_p) and _p not in sys.path:
        sys.path.append(_p)

import numpy as np

B, S, V = 4, 2048, 1024
NH, HD = 16, 64
HPC = 8  # heads per core
PAIRS = HPC // 2
QB = 256  # query block (matmul moving dim)
NQB = S // QB
KT = 128  # key tile
NEG = -1.0e30


def _build_nc():
    import concourse.bacc as bacc
    import concourse.tile as tile
    from concourse import mybir

    F32 = mybir.dt.float32
    F32R = mybir.dt.float32r
    BF16 = mybir.dt.bfloat16
    EXPF = mybir.ActivationFunctionType.Exp

    nc = bacc.Bacc(None, target_bir_lowering=False)

    xT = nc.dram_tensor("xT", [PAIRS * 128, S], BF16, kind="ExternalInput")
    wq = nc.dram_tensor("wq", [128, 128], BF16, kind="ExternalInput")
    wk = nc.dram_tensor("wk", [128, 128], BF16, kind="ExternalInput")
    wv = nc.dram_tensor("wv", [128, 128], BF16, kind="ExternalInput")
    woT = nc.dram_tensor("woT", [PAIRS * 128, V], BF16, kind="ExternalInput")
    maskadd = nc.dram_tensor("maskadd", [128, 128], F32, kind="ExternalInput")
    out = nc.dram_tensor("out", [S, V], F32R, kind="ExternalOutput")

    with tile.TileContext(nc) as tc:
        with (
            tc.tile_pool(name="persist", bufs=1) as persist,
            tc.tile_pool(name="xstage", bufs=4) as xstage,
            tc.tile_pool(name="epool", bufs=12) as epool,
            tc.tile_pool(name="strip", bufs=2) as strip,
            tc.tile_pool(name="small", bufs=6) as small,
            tc.tile_pool(name="rtpool", bufs=2) as rtpool,
            tc.tile_pool(name="bcpool", bufs=4) as bcpool,
            tc.tile_pool(name="outsb", bufs=2) as outsb,
            tc.tile_pool(name="ps_s", bufs=2, space="PSUM") as ps_s,
            tc.tile_pool(name="ps_c", bufs=1, space="PSUM") as ps_c,
            tc.tile_pool(name="ps_o", bufs=1, space="PSUM") as ps_o,
        ):
            # ---- constants ----
            wq_sb = persist.tile([128, 128], BF16, tag="wq")
            wk_sb = persist.tile([128, 128], BF16, tag="wk")
            wv_sb = persist.tile([128, 128], BF16, tag="wv")
            nc.sync.dma_start(wq_sb[:], wq[:])
            nc.sync.dma_start(wk_sb[:], wk[:])
            nc.sync.dma_start(wv_sb[:], wv[:])
            woT_sb = persist.tile([128, PAIRS, V], BF16, tag="wo")
            mask_sb = persist.tile([128, 128], F32, tag="mask")
            nc.sync.dma_start(mask_sb[:], maskadd[:])
            ones_col = persist.tile([128, 2, S // KT, 1], F32, tag="ones")
            nc.vector.memset(ones_col[:], 1.0)

            # ---- phase 1: QKV for all head pairs ----
            qT = []  # per pair [128, S]
            kT_ = []
            vaug = []  # per pair [128, 2, S // KT, 65]
            xh = []  # per pair, two [128, 1024] halves
            for p in range(PAIRS):
                halves = []
                for h in range(2):
                    xt = xstage.tile([128, 1024], BF16, tag="xh")
                    eng = nc.sync if (p + h) % 2 == 0 else nc.scalar
                    eng.dma_start(
                        xt[:], xT[p * 128 : (p + 1) * 128, h * 1024 : (h + 1) * 1024]
                    )
                    halves.append(xt)
                xh.append(halves)
            for p in range(PAIRS):
                nc.sync.dma_start(woT_sb[:, p, :], woT[p * 128 : (p + 1) * 128, :])

            for p in range(PAIRS):
                q_sb = persist.tile([128, S], BF16, tag=f"q{p}")
                k_sb = persist.tile([128, S], BF16, tag=f"k{p}")
                va = persist.tile([128, 2, S // KT, 65], BF16, tag=f"v{p}")
                nc.vector.tensor_copy(out=va[:, :, :, 64:65], in_=ones_col[:])
                for h in range(2):
                    x2 = xh[p][h]
                    # q, k: w stationary, x moving (N=512); v: x chunk
                    # stationary (its LDW hides under the q/k streams)
                    pq = ps_s.tile([128, 1024], F32, tag="s_ps")
                    pk = ps_s.tile([128, 1024], F32, tag="s_ps")
                    pv = ps_c.tile([128, 512], F32, tag="c_ps")
                    pv2 = ps_o.tile([128, 512], F32, tag="o_ps")
                    for i in range(2):
                        vbuf = pv if i == 0 else pv2
                        nc.tensor.matmul(
                            pq[:, i * 512 : (i + 1) * 512],
                            wq_sb[:],
                            x2[:, i * 512 : (i + 1) * 512],
                            start=True,
                            stop=True,
                        )
                        for c in (0, 1):  # s-chunks within this 512 block
                            nc.tensor.matmul(
                                vbuf[:, c * 128 : (c + 1) * 128],
                                x2[:, i * 512 + c * 128 : i * 512 + (c + 1) * 128],
                                wv_sb[:],
                                start=True,
                                stop=True,
                            )
                        nc.tensor.matmul(
                            pk[:, i * 512 : (i + 1) * 512],
                            wk_sb[:],
                            x2[:, i * 512 : (i + 1) * 512],
                            start=True,
                            stop=True,
                        )
                        for c in (2, 3):
                            nc.tensor.matmul(
                                vbuf[:, c * 128 : (c + 1) * 128],
                                x2[:, i * 512 + c * 128 : i * 512 + (c + 1) * 128],
                                wv_sb[:],
                                start=True,
                                stop=True,
                            )
                    nc.scalar.copy(q_sb[:, h * 1024 : (h + 1) * 1024], pq[:])
                    nc.scalar.copy(k_sb[:, h * 1024 : (h + 1) * 1024], pk[:])
                    # vbuf slot c of pv holds s-chunk c, pv2 holds 4+c
                    for buf, off in ((pv, 0), (pv2, 4)):
                        for c in range(4):
                            nc.vector.tensor_copy(
                                out=va[:, :, h * 8 + off + c, 0:64],
                                in_=buf[:, c * 128 : (c + 1) * 128].rearrange(
                                    "p (two c) -> p two c", two=2
                                ),
                            )
                qT.append(q_sb)
                kT_.append(k_sb)
                vaug.append(va)

            # ---- phase 2 ----
            # Deferred-work queue: closures issuing ctx / normalize for the
            # previous pair and output-projection chunks for the previous
            # q-block, drained between scores groups to keep every engine fed.
            work = deque()
            op_work = deque()

            def drain(n):
                for _ in range(min(n, len(work))):
                    work.popleft()()

            def drain_op(n):
                for _ in range(min(n, len(op_work))):
                    op_work.popleft()()

            def make_ctx(p, qb, e_tiles, cstrip, den8, rt8):
                q0 = qb * QB
                nkt = (q0 + QB) // KT
                cps = ps_c.tile([65, 512], F32, tag="c_ps")

                def ctx_group(hh, g):
                    # one head's staging-group chunk; heads strictly
                    # sequenced (A fully accumulated before B starts): a
                    # start=True clears the shared bank's has_written bits
                    def run():
                        e_sb = e_tiles[g]
                        for j, off, w in groups[g]:
                            rhs = e_sb[:, hh, off : off + w]
                            outp = cps[:, hh * 256 + QB - w : (hh + 1) * 256]
                            nc.tensor.matmul(
                                outp,
                                vaug[p][:, hh, j, :],
                                rhs,
                                start=(j == 0),
                                stop=(j == nkt - 1),
                                skip_group_check=True,
                            )

                    return run

                def stage_out():
                    # unnormalized ctx -> cstrip (frees the PSUM bank); both
                    # heads' denominator rows -> srow (aligned) -> one DMA
                    # into den8[2p:2p+2]
                    srow = small.tile([65, 2 * QB], F32, tag="srow")
                    nc.vector.tensor_copy(out=srow[64:65, :], in_=cps[64:65, :])
                    nc.sync.dma_start(
                        den8[2 * p : 2 * p + 2, :],
                        srow[64:65, :].rearrange("o (hh q) -> o hh q", hh=2),
                    )
                    for hh in range(2):
                        nc.vector.tensor_copy(
                            out=cstrip[hh * 64 : hh * 64 + 64, p, :],
                            in_=cps[0:64, hh * 256 : hh * 256 + 256],
                        )

                def norm():
                    for hh in range(2):
                        bc = bcpool.tile([128, QB], F32, tag="bc")
                        nc.gpsimd.partition_broadcast(
                            bc[:], rt8[0:1, 2 * p + hh, :], channels=128
                        )
                        nc.vector.tensor_mul(
                            cstrip[hh * 64 : hh * 64 + 64, p, :],
                            cstrip[hh * 64 : hh * 64 + 64, p, :],
                            bc[hh * 64 : hh * 64 + 64, :],
                        )

                for hh in range(2):
                    for g in range(len(groups)):
                        work.append(ctx_group(hh, g))
                work.append(stage_out)
                return norm

            def make_outproj(cstrip, q0, final=False):
                def chunk(st, oc):
                    def run():
                        ops = ps_o.tile([128, 512], F32, tag="o_ps")
                        for p in range(PAIRS):
                            nc.tensor.matmul(
                                ops[:],
                                cstrip[:, p, st * 128 : (st + 1) * 128],
                                woT_sb[:, p, oc * 512 : (oc + 1) * 512],
                                start=(p == 0),
                                stop=(p == PAIRS - 1),
                            )
                        o_sb = outsb.tile([128, 512], F32R, tag="osb")
                        if final:
                            nc.scalar.copy(o_sb[:], ops[:])
                        else:
                            nc.vector.tensor_copy(out=o_sb[:], in_=ops[:])
                        nc.sync.dma_start(
                            out[q0 + st * 128 : q0 + (st + 1) * 128,
                                oc * 512 : (oc + 1) * 512],
                            o_sb[:],
                        )

                    return run

                for st in range(QB // 128):
                    for oc in range(V // 512):
                        op_work.append(chunk(st, oc))

            prev_cstrip = None
            prev_q0 = None
            for qb in range(NQB):
                q0 = qb * QB
                nkt = (q0 + QB) // KT
                # pack k-tiles into staging groups of <=3; the final tile is
                # compacted to its valid 128 columns
                groups = []
                for j0 in range(0, nkt, 3):
                    tiles, off = [], 0
                    for j in range(j0, min(j0 + 3, nkt)):
                        w = 128 if j == nkt - 1 else 256
                        tiles.append((j, off, w))
                        off += w
                    groups.append(tiles)
                tile_pos = {j: (g, off, w) for g, ts in enumerate(groups)
                            for j, off, w in ts}
                cstrip = strip.tile([128, PAIRS, QB], BF16, tag="cstrip")
                den8 = small.tile([HPC, QB], F32, tag="den8")
                rec8 = small.tile([HPC, QB], F32, tag="rec8")
                rt8 = rtpool.tile([1, HPC, QB], F32, tag="rt8")
                norms = []
                if prev_cstrip is not None:
                    make_outproj(prev_cstrip, prev_q0)
                for p in range(PAIRS):
                    e_tiles = []
                    for g, tiles in enumerate(groups):
                        bw = sum(w for _, _, w in tiles)  # block width per hh
                        sps = ps_s.tile([128, 2, 768], F32, tag="s_ps")
                        for j, off, w in tiles:
                            for hh in range(2):  # adjacent A/B: row-tiled
                                r0 = hh * 64
                                nc.tensor.matmul(
                                    sps[:, hh, off : off + w],
                                    kT_[p][r0 : r0 + 64, j * KT : (j + 1) * KT],
                                    qT[p][r0 : r0 + 64, q0 + QB - w : q0 + QB],
                                    start=True,
                                    stop=True,
                                )
                        e_sb = epool.tile([128, 2, 768], BF16, tag="e")
                        for j, off, w in tiles:
                            # causal masks on the two final k-tiles
                            if j >= nkt - 2:
                                moff = off
                                nc.vector.tensor_add(
                                    sps[:, :, moff : moff + 128],
                                    sps[:, :, moff : moff + 128],
                                    mask_sb[:, None, :].to_broadcast((128, 2, 128)),
                                )
                        nc.scalar.activation(
                            out=e_sb[:, :, 0:bw],
                            in_=sps[:, :, 0:bw],
                            func=EXPF,
                            scale=0.125,
                        )
                        e_tiles.append(e_sb)
                        if g < len(groups) - 1:
                            drain(2)
                    norms.append(make_ctx(p, qb, e_tiles, cstrip, den8, rt8))
                    drain(4)
                    if p >= 2:
                        drain_op(2)
                # all pairs' denominators gathered -> one batched reciprocal,
                # then the per-head broadcast+multiply closures
                drain(len(work))
                nc.vector.reciprocal_approx_fast(out=rec8[:], in_=den8[:])
                nc.sync.dma_start(rt8[0:1, :, :], rec8[:, :])
                for nrm in norms:
                    nrm()
                prev_cstrip = cstrip
                prev_q0 = q0
            make_outproj(prev_cstrip, prev_q0, final=True)
            drain(len(work))
            drain_op(len(op_work))
    nc.compile()
    return nc


_NC = None


def _get_nc():
    global _NC
    if _NC is None:
        _NC = _build_nc()
    return _NC


def _host_inputs(x, w_qkv, w_out):
    """Build the 8 per-core input maps from the full tensors."""
    import ml_dtypes

    BF = ml_dtypes.bfloat16
    x = np.asarray(x, dtype=np.float32)
    w_qkv = np.asarray(w_qkv, dtype=np.float32)
    w_out = np.asarray(w_out, dtype=np.float32)

    def blockdiag(m):  # m [64, 64] -> [128, 128]
        z = np.zeros((128, 128), dtype=np.float32)
        z[0:64, 0:64] = m
        z[64:128, 64:128] = m
        return z

    wq = blockdiag(w_qkv[0:64].T.copy())
    wk = blockdiag(w_qkv[64:128].T.copy())
    wv = blockdiag(w_qkv[128:192].T.copy())

    kk = np.arange(128)[:, None]
    qq = np.arange(128)[None, :]
    maskadd = np.where(kk <= qq, 0.0, NEG).astype(np.float32)

    in_maps = []
    for c in range(8):
        b, g = c // 2, c % 2
        cols = slice(g * 512, (g + 1) * 512)
        in_maps.append(
            {
                "xT": np.ascontiguousarray(x[b][:, cols].T).astype(BF),
                "wq": wq.astype(BF),
                "wk": wk.astype(BF),
                "wv": wv.astype(BF),
                "woT": np.ascontiguousarray(w_out[:, cols].T).astype(BF),
                "maskadd": maskadd,
            }
        )
    return in_maps


def run(x, w_qkv, w_out, trace=False, tmpdir=None):
    from concourse.bass_utils import run_bass_kernel_spmd

    nc = _get_nc()
    in_maps = _host_inputs(x, w_qkv, w_out)
    res = run_bass_kernel_spmd(
        nc, in_maps, core_ids=list(range(8)), trace=trace, tmpdir=tmpdir
    )
    outs = [r["out"] for r in res.results]
    full = np.empty((B, S, V), dtype=np.float32)
    for b in range(B):
        full[b] = outs[2 * b] + outs[2 * b + 1]
    return full, res


def kernel(x, w_qkv, w_out):
    full, _ = run(x, w_qkv, w_out)
    return full
